# revision 2
# baseline (speedup 1.0000x reference)
"""GCN 2-layer message-passing block on 8 Trainium2 NeuronCores.

Collapsed algebra (validated against the jax reference to 7e-7 in fp64):
  dis = deg^-0.5 (deg over edge sources), x~ = dis * x          (host)
  a[d] = sum_{e->d} dis[row_e];  c = dis*a                      (host)
  c2   = dis * A(dis*c);  W12 = W2@W1;  v = W2@b1               (host)
  g1[u]  = sum_{e: col=u} x~[row_e]      -- aggregation 1 (device)
  tab1   = dis^2 * g1                    -- scale (device)
  g2[d]  = sum_{e->d} tab1[row_e]        -- aggregation 2 (device)
  y2     = (dis*g2) @ W12.T + c2 (x) v + c (x) b2   -- one Lin + rank-1s

Sharding: destinations split into 8 blocks of 12500 nodes; zero-communication
(each core re-derives tab1 at S_c = distinct sources of its own edges).
Aggregations are matmuls with static one-hot block matrices (fp8) in fp32
PSUM; the inter-stage shuffle uses the custom dma_gather.  Destination tiles
are bin-packed by in-degree so every tile needs exactly 2 token blocks.

v2: bf16 output (host casts to f32), partition-major DRAM layouts for the
tab1 intermediate and the output (contiguous per-partition descriptors),
DMA batched 8 stage-A tiles / 10 stage-B tiles per call, gathers batched
10 tiles per call to amortize the SWDGE fixed cost.
"""
import sys

sys.path.insert(0, "/opt/trn_rl_repo")

import numpy as np
import ml_dtypes

BF16 = ml_dtypes.bfloat16
FP8 = ml_dtypes.float8_e4m3

N_NODES = 100000
N_EDGES = 200000
H = 384
KB = H // 128
M_CORES = 8
NPC = N_NODES // M_CORES
B = 2                 # token blocks per dest tile (bin-packed)
CAP = B * 128         # 256 token slots per tile
QL = 8                # stage-A tiles per load/store group
SG = 10               # stage-B tiles per gather/store group


def _ffdpack(sizes):
    """Bin-pack dests into tiles of <=128 dests and <=CAP tokens, largest
    sizes first, vectorized per size class.  Returns (tile_of, slot_of,
    tok_off, ntiles): tile, within-tile dest index, within-tile token
    offset for every dest."""
    n = sizes.size
    nb = max(int(np.ceil(sizes.sum() / (CAP - 2))), int(np.ceil(n / 128)))
    while True:
        rem_tok = np.full(nb, CAP, np.int64)
        rem_cnt = np.full(nb, 128, np.int64)
        tile_of = np.full(n, -1, np.int64)
        ok = True
        for s in range(int(sizes.max()), -1, -1):
            items = np.nonzero(sizes == s)[0]
            ii = 0
            while ii < items.size:
                elig = np.nonzero((rem_tok >= s) & (rem_cnt > 0))[0]
                if elig.size == 0:
                    ok = False
                    break
                # fill most-empty bins first
                elig = elig[np.argsort(-rem_tok[elig], kind="stable")]
                take = min(items.size - ii, elig.size)
                sel = elig[:take]
                tile_of[items[ii:ii + take]] = sel
                rem_tok[sel] -= s
                rem_cnt[sel] -= 1
                ii += take
            if not ok:
                break
        if ok:
            break
        nb += 1
    # within-tile ordering: by dest id for determinism
    order = np.lexsort((np.arange(n), tile_of))
    slot_of = np.empty(n, np.int64)
    tok_off = np.empty(n, np.int64)
    cnt = np.zeros(nb, np.int64)
    tok = np.zeros(nb, np.int64)
    for i in order:
        t = tile_of[i]
        slot_of[i] = cnt[t]
        tok_off[i] = tok[t]
        cnt[t] += 1
        tok[t] += sizes[i]
    return tile_of, slot_of, tok_off, nb


def _wrap_idx(idx):
    n = idx.size
    w = idx.reshape(n // 16, 16).T.astype(np.int16)
    return np.tile(w, (8, 1))


def _pm_groups(tokens, ntiles, width, dtype, Q):
    """[ntiles*CAP, width] -> group-major [ntiles//Q, 128, Q*B, width]."""
    t = tokens.reshape(ntiles // Q, Q, B, 128, width)
    return np.ascontiguousarray(t.transpose(0, 3, 1, 2, 4).reshape(
        ntiles // Q, 128, Q * B, width)).astype(dtype)


def _prep(x, edge_index, W1, b1, W2, b2):
    row = np.asarray(edge_index[0], dtype=np.int64)
    col = np.asarray(edge_index[1], dtype=np.int64)
    xf = np.asarray(x, dtype=np.float64)

    deg = np.bincount(row, minlength=N_NODES).astype(np.float64)
    dis = deg ** -0.5
    a = np.bincount(col, weights=dis[row], minlength=N_NODES)
    cvec = dis * a
    c2 = dis * np.bincount(col, weights=(dis * cvec)[row], minlength=N_NODES)
    W12 = np.asarray(W2, np.float64) @ np.asarray(W1, np.float64)
    vv = np.asarray(W2, np.float64) @ np.asarray(b1, np.float64)
    xt = (dis[:, None] * xf).astype(BF16)

    core_of = col // NPC

    # ---- per-core packing metadata; uniform sizes across cores ----
    metas = []
    NTA = 0
    NTB = 0
    for cc in range(M_CORES):
        em = core_of == cc
        er, ec = row[em], col[em]
        S = np.unique(er)
        pos = np.full(N_NODES, -1, dtype=np.int64)
        pos[S] = np.arange(S.size)
        e2m = pos[col] >= 0
        dc2, r2 = pos[col[e2m]], row[e2m]          # stage-A tokens
        szA = np.bincount(dc2, minlength=S.size)
        tileA, slotA_d, tokoffA, ntA = _ffdpack(szA)
        dlo = ec - cc * NPC
        szB = np.bincount(dlo, minlength=NPC)
        tileB, slotB_d, tokoffB, ntB = _ffdpack(szB)
        NTA = max(NTA, ntA)
        NTB = max(NTB, ntB)
        metas.append((er, ec, S, pos, dc2, r2, szA, tileA, slotA_d, tokoffA,
                      dlo, szB, tileB, slotB_d, tokoffB))
    NTA += (-NTA) % QL        # multiple of the stage-A group
    NTB += (-NTB) % SG        # multiple of the stage-B group
    assert NTA * 128 < 2 ** 15  # gather row ids are int16

    w12 = np.ascontiguousarray(
        W12.T.astype(BF16).reshape(KB, 128, H).transpose(1, 0, 2))
    vrow = vv.astype(BF16).reshape(1, H)
    b2r = np.asarray(b2, dtype=BF16).reshape(1, H)

    in_maps = []
    perms = []
    for cc in range(M_CORES):
        (er, ec, S, pos, dc2, r2, szA, tileA, slotA_d, tokoffA,
         dlo, szB, tileB, slotB_d, tokoffB) = metas[cc]
        nA = NTA * CAP
        nB = NTB * CAP

        # token slot for each stage-A edge: tile base + within-tile offset
        offA = np.zeros(S.size + 1, np.int64)
        np.cumsum(szA, out=offA[1:])
        order2 = np.argsort(dc2, kind="stable")
        rank2 = np.arange(dc2.size) - offA[dc2[order2]]
        dst = dc2[order2]
        slotA = tileA[dst] * CAP + tokoffA[dst] + rank2
        t1 = np.zeros((nA, H), dtype=BF16)
        t1[slotA] = xt[r2[order2]]
        s1 = np.zeros((nA, 128), dtype=np.float32)
        s1[slotA, slotA_d[dst]] = 1.0

        tmp = np.zeros(NTA * 128, dtype=np.float32)
        tmp[tileA[np.arange(S.size)] * 128 + slotA_d] = (
            dis[S] ** 2).astype(np.float32)
        dis2arr = np.ascontiguousarray(tmp.reshape(NTA, 128).T)
        # tab1 row of source u in the partition-major [128, NTA, H] layout
        tabrow = slotA_d * NTA + tileA  # indexed by compact id

        # ---- stage B ----
        offB = np.zeros(NPC + 1, np.int64)
        np.cumsum(szB, out=offB[1:])
        orderB = np.argsort(dlo, kind="stable")
        rankB = np.arange(dlo.size) - offB[dlo[orderB]]
        dstB = dlo[orderB]
        slotB = tileB[dstB] * CAP + tokoffB[dstB] + rankB
        gidx = np.zeros(nB, dtype=np.int64)
        gidx[slotB] = tabrow[pos[er[orderB]]]
        s2 = np.zeros((nB, 128), dtype=np.float32)
        s2[slotB, slotB_d[dstB]] = 1.0

        # per-owned-node indices: c12 columns are tile*128+slot; the
        # bf16 output is partition-major so rows are slot*NTB+tile
        pown = tileB * 128 + slotB_d
        pout = slotB_d * NTB + tileB

        tmp = np.zeros(NTB * 128, dtype=np.float32)
        tmp[pown] = dis[cc * NPC:(cc + 1) * NPC].astype(np.float32)
        disarr = np.ascontiguousarray(tmp.reshape(NTB, 128).T)
        dd = dis[cc * NPC:(cc + 1) * NPC]
        c12 = np.zeros((2, NTB * 128), dtype=BF16)
        c12[0, pown] = (c2[cc * NPC:(cc + 1) * NPC] / dd).astype(BF16)
        c12[1, pown] = (cvec[cc * NPC:(cc + 1) * NPC] / dd).astype(BF16)

        in_maps.append({
            "t1": _pm_groups(t1, NTA, H, BF16, QL),
            "s1": _pm_groups(s1, NTA, 128, FP8, QL),
            "dis2": dis2arr,
            "gidx": _wrap_idx(gidx),
            "s2": _pm_groups(s2, NTB, 128, FP8, SG),
            "disc": disarr,
            "c12": c12,
            "w12": w12, "vb2": np.concatenate([vrow, b2r], axis=0),
        })
        perms.append(pout)
    return in_maps, dict(NTA=NTA, NTB=NTB), perms


def _build(dims):
    import concourse.bass as bass
    import concourse.bacc as bacc
    import concourse.mybir as mybir
    import concourse.tile as tile

    dt = mybir.dt
    AF = mybir.ActivationFunctionType
    NTA, NTB = dims["NTA"], dims["NTB"]
    nB = NTB * CAP
    ND = NTB * 128

    nc = bacc.Bacc(None, target_bir_lowering=False)
    t1 = nc.dram_tensor("t1", [NTA // QL, 128, QL * B, H], dt.bfloat16, kind="ExternalInput")
    s1 = nc.dram_tensor("s1", [NTA // QL, 128, QL * B, 128], dt.float8e4, kind="ExternalInput")
    dis2 = nc.dram_tensor("dis2", [128, NTA], dt.float32, kind="ExternalInput")
    gidx = nc.dram_tensor("gidx", [128, nB // 16], dt.int16, kind="ExternalInput")
    s2 = nc.dram_tensor("s2", [NTB // SG, 128, SG * B, 128], dt.float8e4, kind="ExternalInput")
    disc = nc.dram_tensor("disc", [128, NTB], dt.float32, kind="ExternalInput")
    c12 = nc.dram_tensor("c12", [2, ND], dt.bfloat16, kind="ExternalInput")
    w12 = nc.dram_tensor("w12", [128, KB, H], dt.bfloat16, kind="ExternalInput")
    vb2 = nc.dram_tensor("vb2", [2, H], dt.bfloat16, kind="ExternalInput")
    tab1 = nc.dram_tensor("tab1", [128, NTA, H], dt.bfloat16, kind="Internal")
    out = nc.dram_tensor("out", [128, NTB, H], dt.bfloat16, kind="ExternalOutput")

    with tile.TileContext(nc) as tc:
        with (
            tc.tile_pool(name="const", bufs=1) as cp,
            tc.tile_pool(name="io", bufs=3) as iop,
            tc.tile_pool(name="stg", bufs=3) as stgp,
            tc.tile_pool(name="ps", bufs=2, space="PSUM") as psp,
        ):
            w12_sb = cp.tile([128, KB, H], dt.bfloat16)
            nc.sync.dma_start(w12_sb[:], w12[:])
            vb2_sb = cp.tile([2, H], dt.bfloat16)
            nc.sync.dma_start(vb2_sb[:], vb2[:])
            dis2_sb = cp.tile([128, NTA], dt.float32)
            nc.sync.dma_start(dis2_sb[:], dis2[:])
            disc_sb = cp.tile([128, NTB], dt.float32)
            nc.sync.dma_start(disc_sb[:], disc[:])
            gidx_sb = cp.tile([128, nB // 16], dt.int16)
            nc.sync.dma_start(gidx_sb[:], gidx[:])
            c12_sb = cp.tile([2, ND], dt.bfloat16)
            nc.gpsimd.dma_start(c12_sb[:], c12[:])

            # ---------- stage A: tab1 = dis^2 * (S1 @ t1) ----------
            for p in range(NTA // QL):
                t1_sb = iop.tile([128, QL * B, H], dt.bfloat16, tag="t1")
                nc.sync.dma_start(t1_sb[:], t1[p])
                s1_sb = iop.tile([128, QL * B, 128], dt.float8e4, tag="s1")
                nc.vector.dma_start(s1_sb[:], s1[p])
                tws = stgp.tile([128, QL, H], dt.bfloat16, tag="tws")
                for h in range(QL):
                    i = p * QL + h
                    psA = psp.tile([128, H], dt.float32, tag="psA")
                    for b in range(B):
                        nc.tensor.matmul(psA[:], s1_sb[:, h * B + b, :],
                                         t1_sb[:, h * B + b, :],
                                         start=(b == 0), stop=(b == B - 1))
                    if i % 2 == 0:
                        nc.vector.tensor_scalar_mul(
                            tws[:, h, :], psA[:], dis2_sb[:, i:i + 1])
                    else:
                        nc.scalar.activation(
                            tws[:, h, :], psA[:], AF.Copy,
                            scale=dis2_sb[:, i:i + 1])
                nc.sync.dma_start(tab1[:, p * QL:(p + 1) * QL, :], tws[:])

            # ------- stages B/C/D fused per owned-tile group ----------
            tab1f = tab1[:].rearrange("p t h -> (p t) h")
            for q in range(NTB // SG):
                s2_sb = iop.tile([128, SG * B, 128], dt.float8e4, tag="s2")
                nc.scalar.dma_start(s2_sb[:], s2[q])
                g_sb = iop.tile([128, SG * B, H], dt.bfloat16, tag="g")
                nc.gpsimd.dma_gather(
                    g_sb[:], tab1f,
                    gidx_sb[:, q * (SG * CAP // 16):(q + 1) * (SG * CAP // 16)],
                    SG * CAP, SG * CAP, H, transpose=False)
                ows = stgp.tile([128, SG, H], dt.bfloat16, tag="ows")
                for h in range(SG):
                    j = q * SG + h
                    psC = psp.tile([128, H], dt.float32, tag="psC")
                    for fs in range(KB):
                        for b in range(B):
                            nc.tensor.matmul(
                                psC[:, fs * 128:(fs + 1) * 128],
                                g_sb[:, h * B + b, fs * 128:(fs + 1) * 128],
                                s2_sb[:, h * B + b, :],
                                start=(b == 0), stop=(b == B - 1))
                    zf = iop.tile([128, H], dt.bfloat16, tag="zf")
                    if j % 2 == 0:
                        nc.vector.tensor_copy(zf[:], psC[:])
                    else:
                        nc.scalar.activation(zf[:], psC[:], AF.Copy)
                    psD = psp.tile([128, H], dt.float32, tag="psD")
                    for k in range(KB):
                        nc.tensor.matmul(psD[:], zf[:, k * 128:(k + 1) * 128],
                                         w12_sb[:, k, :],
                                         start=(k == 0), stop=False)
                    nc.tensor.matmul(psD[:], c12_sb[:, j * 128:(j + 1) * 128],
                                     vb2_sb[:], start=False, stop=True)
                    if j % 2 == 0:
                        nc.vector.tensor_scalar_mul(ows[:, h, :], psD[:],
                                                    disc_sb[:, j:j + 1])
                    else:
                        nc.scalar.activation(ows[:, h, :], psD[:], AF.Copy,
                                             scale=disc_sb[:, j:j + 1])
                nc.sync.dma_start(out[:, q * SG:(q + 1) * SG, :], ows[:])
    nc.compile()
    return nc


_CACHE = {}


def kernel(x, edge_index, W1, b1, W2, b2):
    from concourse import bass_utils

    in_maps, dims, perms = _prep(x, edge_index, W1, b1, W2, b2)
    key = tuple(sorted(dims.items()))
    if key not in _CACHE:
        _CACHE[key] = _build(dims)
    nc = _CACHE[key]
    res = bass_utils.run_bass_kernel_spmd(nc, in_maps, core_ids=list(range(M_CORES)))
    NTB = dims["NTB"]
    out = np.empty((N_NODES, H), np.float32)
    for cc in range(M_CORES):
        flat = np.asarray(res.results[cc]["out"]).reshape(128 * NTB, H)
        out[cc * NPC:(cc + 1) * NPC] = flat[perms[cc]].astype(np.float32)
    return out


# revision 10
# speedup vs baseline: 1.8238x; 1.8238x over previous
"""GCN 2-layer message-passing block on 8 Trainium2 NeuronCores.

Collapsed algebra (validated against the jax reference to 7e-7 in fp64):
  dis = deg^-0.5 (deg over edge sources), x~ = dis * x          (host)
  a[d] = sum_{e->d} dis[row_e];  c = dis*a                      (host)
  c2   = dis * A(dis*c);  W12 = W2@W1;  v = W2@b1               (host)
  g1[u]  = sum_{e: col=u} x~[row_e]      -- aggregation 1 (device)
  tab1   = dis^2 * g1                    -- scale (device)
  g2[d]  = sum_{e->d} tab1[row_e]        -- aggregation 2 (device)
  y2     = (dis*g2) @ W12.T + c2 (x) v + c (x) b2   -- one Lin + rank-1s

Sharding: destinations split into 8 blocks of 12500 nodes; zero-communication
(each core re-derives tab1 at S_c = distinct sources of its own edges).
Aggregations are matmuls with static one-hot block matrices (fp8) in fp32
PSUM; the inter-stage shuffle uses the custom dma_gather.  Destination tiles
are bin-packed by in-degree so every tile needs exactly 2 token blocks.

v2: bf16 output (host casts to f32), partition-major DRAM layouts for the
tab1 intermediate and the output (contiguous per-partition descriptors),
DMA batched 8 stage-A tiles / 10 stage-B tiles per call, gathers batched
10 tiles per call to amortize the SWDGE fixed cost.
"""
import sys

sys.path.insert(0, "/opt/trn_rl_repo")

import numpy as np
import ml_dtypes

BF16 = ml_dtypes.bfloat16
FP8 = ml_dtypes.float8_e4m3

N_NODES = 100000
N_EDGES = 200000
H = 384
KB = H // 128
M_CORES = 8
NPC = N_NODES // M_CORES
B = 2                 # token blocks per dest tile (bin-packed)
CAP = B * 128         # 256 token slots per tile
QL = 8                # stage-A tiles per load/store group
SG = 10               # stage-B tiles per gather/store group


def _ffdpack(sizes):
    """Bin-pack dests into tiles of <=128 dests and <=CAP tokens, largest
    sizes first, vectorized per size class.  Returns (tile_of, slot_of,
    tok_off, ntiles): tile, within-tile dest index, within-tile token
    offset for every dest."""
    n = sizes.size
    nb = max(int(np.ceil(sizes.sum() / (CAP - 2))), int(np.ceil(n / 128)))
    while True:
        rem_tok = np.full(nb, CAP, np.int64)
        rem_cnt = np.full(nb, 128, np.int64)
        tile_of = np.full(n, -1, np.int64)
        ok = True
        for s in range(int(sizes.max()), -1, -1):
            items = np.nonzero(sizes == s)[0]
            ii = 0
            while ii < items.size:
                elig = np.nonzero((rem_tok >= s) & (rem_cnt > 0))[0]
                if elig.size == 0:
                    ok = False
                    break
                # fill most-empty bins first
                elig = elig[np.argsort(-rem_tok[elig], kind="stable")]
                take = min(items.size - ii, elig.size)
                sel = elig[:take]
                tile_of[items[ii:ii + take]] = sel
                rem_tok[sel] -= s
                rem_cnt[sel] -= 1
                ii += take
            if not ok:
                break
        if ok:
            break
        nb += 1
    # within-tile ordering: by dest id for determinism
    order = np.lexsort((np.arange(n), tile_of))
    slot_of = np.empty(n, np.int64)
    tok_off = np.empty(n, np.int64)
    cnt = np.zeros(nb, np.int64)
    tok = np.zeros(nb, np.int64)
    for i in order:
        t = tile_of[i]
        slot_of[i] = cnt[t]
        tok_off[i] = tok[t]
        cnt[t] += 1
        tok[t] += sizes[i]
    return tile_of, slot_of, tok_off, nb


def _wrap_idx(idx):
    n = idx.size
    w = idx.reshape(n // 16, 16).T.astype(np.int16)
    return np.tile(w, (8, 1))


def _pm_groups(tokens, ntiles, width, dtype, Q):
    """[ntiles*CAP, width] -> group-major [ntiles//Q, 128, Q*B, width]."""
    t = tokens.reshape(ntiles // Q, Q, B, 128, width)
    return np.ascontiguousarray(t.transpose(0, 3, 1, 2, 4).reshape(
        ntiles // Q, 128, Q * B, width)).astype(dtype)


def _prep(x, edge_index, W1, b1, W2, b2):
    row = np.asarray(edge_index[0], dtype=np.int64)
    col = np.asarray(edge_index[1], dtype=np.int64)
    xf = np.asarray(x, dtype=np.float64)

    deg = np.bincount(row, minlength=N_NODES).astype(np.float64)
    dis = deg ** -0.5
    a = np.bincount(col, weights=dis[row], minlength=N_NODES)
    cvec = dis * a
    c2 = dis * np.bincount(col, weights=(dis * cvec)[row], minlength=N_NODES)
    W12 = np.asarray(W2, np.float64) @ np.asarray(W1, np.float64)
    vv = np.asarray(W2, np.float64) @ np.asarray(b1, np.float64)
    xt = (dis[:, None] * xf).astype(BF16)

    core_of = col // NPC

    # ---- per-core packing metadata; uniform sizes across cores ----
    metas = []
    NTA = 0
    NTB = 0
    for cc in range(M_CORES):
        em = core_of == cc
        er, ec = row[em], col[em]
        S = np.unique(er)
        pos = np.full(N_NODES, -1, dtype=np.int64)
        pos[S] = np.arange(S.size)
        e2m = pos[col] >= 0
        dc2, r2 = pos[col[e2m]], row[e2m]          # stage-A tokens
        szA = np.bincount(dc2, minlength=S.size)
        tileA, slotA_d, tokoffA, ntA = _ffdpack(szA)
        dlo = ec - cc * NPC
        szB = np.bincount(dlo, minlength=NPC)
        tileB, slotB_d, tokoffB, ntB = _ffdpack(szB)
        NTA = max(NTA, ntA)
        NTB = max(NTB, ntB)
        metas.append((er, ec, S, pos, dc2, r2, szA, tileA, slotA_d, tokoffA,
                      dlo, szB, tileB, slotB_d, tokoffB))
    NTA += (-NTA) % QL        # multiple of the stage-A group
    NTB += (-NTB) % SG        # multiple of the stage-B group
    assert NTA * 128 < 2 ** 15  # gather row ids are int16

    w12 = np.ascontiguousarray(
        W12.T.astype(BF16).reshape(KB, 128, H).transpose(1, 0, 2))
    vrow = vv.astype(BF16).reshape(1, H)
    b2r = np.asarray(b2, dtype=BF16).reshape(1, H)

    in_maps = []
    perms = []
    for cc in range(M_CORES):
        (er, ec, S, pos, dc2, r2, szA, tileA, slotA_d, tokoffA,
         dlo, szB, tileB, slotB_d, tokoffB) = metas[cc]
        nA = NTA * CAP
        nB = NTB * CAP

        # token slot for each stage-A edge: tile base + within-tile offset
        offA = np.zeros(S.size + 1, np.int64)
        np.cumsum(szA, out=offA[1:])
        order2 = np.argsort(dc2, kind="stable")
        rank2 = np.arange(dc2.size) - offA[dc2[order2]]
        dst = dc2[order2]
        slotA = tileA[dst] * CAP + tokoffA[dst] + rank2
        t1 = np.zeros((nA, H), dtype=BF16)
        t1[slotA] = xt[r2[order2]]
        s1 = np.zeros((nA, 128), dtype=np.float32)
        s1[slotA, slotA_d[dst]] = 1.0

        tmp = np.zeros(NTA * 128, dtype=np.float32)
        tmp[tileA[np.arange(S.size)] * 128 + slotA_d] = (
            dis[S] ** 2).astype(np.float32)
        dis2arr = np.ascontiguousarray(tmp.reshape(NTA, 128).T)
        # tab1 row of source u in the partition-major [128, NTA, H] layout
        tabrow = slotA_d * NTA + tileA  # indexed by compact id

        # ---- stage B ----
        offB = np.zeros(NPC + 1, np.int64)
        np.cumsum(szB, out=offB[1:])
        orderB = np.argsort(dlo, kind="stable")
        rankB = np.arange(dlo.size) - offB[dlo[orderB]]
        dstB = dlo[orderB]
        slotB = tileB[dstB] * CAP + tokoffB[dstB] + rankB
        gidx = np.zeros(nB, dtype=np.int64)
        gidx[slotB] = tabrow[pos[er[orderB]]]
        s2 = np.zeros((nB, 128), dtype=np.float32)
        s2[slotB, slotB_d[dstB]] = 1.0

        # per-owned-node indices: c12 columns are tile*128+slot; the
        # bf16 output is partition-major so rows are slot*NTB+tile
        pown = tileB * 128 + slotB_d
        pout = slotB_d * NTB + tileB

        tmp = np.zeros(NTB * 128, dtype=np.float32)
        tmp[pown] = dis[cc * NPC:(cc + 1) * NPC].astype(np.float32)
        disarr = np.ascontiguousarray(tmp.reshape(NTB, 128).T)
        dd = dis[cc * NPC:(cc + 1) * NPC]
        c12 = np.zeros((2, NTB * 128), dtype=BF16)
        c12[0, pown] = (c2[cc * NPC:(cc + 1) * NPC] / dd).astype(BF16)
        c12[1, pown] = (cvec[cc * NPC:(cc + 1) * NPC] / dd).astype(BF16)

        in_maps.append({
            "t1": _pm_groups(t1, NTA, H, BF16, QL),
            "s1": _pm_groups(s1, NTA, 128, FP8, QL),
            "dis2": dis2arr,
            "gidx": _wrap_idx(gidx),
            "s2": _pm_groups(s2, NTB, 128, FP8, SG),
            "disc": disarr,
            "c12": c12,
            "w12": w12, "vb2": np.concatenate([vrow, b2r], axis=0),
        })
        perms.append(pout)
    return in_maps, dict(NTA=NTA, NTB=NTB), perms


def _build(dims):
    import concourse.bass as bass
    import concourse.bacc as bacc
    import concourse.mybir as mybir
    import concourse.tile as tile

    dt = mybir.dt
    AF = mybir.ActivationFunctionType
    NTA, NTB = dims["NTA"], dims["NTB"]
    nB = NTB * CAP
    ND = NTB * 128

    nc = bacc.Bacc(None, target_bir_lowering=False)
    t1 = nc.dram_tensor("t1", [NTA // QL, 128, QL * B, H], dt.bfloat16, kind="ExternalInput")
    s1 = nc.dram_tensor("s1", [NTA // QL, 128, QL * B, 128], dt.float8e4, kind="ExternalInput")
    dis2 = nc.dram_tensor("dis2", [128, NTA], dt.float32, kind="ExternalInput")
    gidx = nc.dram_tensor("gidx", [128, nB // 16], dt.int16, kind="ExternalInput")
    s2 = nc.dram_tensor("s2", [NTB // SG, 128, SG * B, 128], dt.float8e4, kind="ExternalInput")
    disc = nc.dram_tensor("disc", [128, NTB], dt.float32, kind="ExternalInput")
    c12 = nc.dram_tensor("c12", [2, ND], dt.bfloat16, kind="ExternalInput")
    w12 = nc.dram_tensor("w12", [128, KB, H], dt.bfloat16, kind="ExternalInput")
    vb2 = nc.dram_tensor("vb2", [2, H], dt.bfloat16, kind="ExternalInput")
    tab1 = nc.dram_tensor("tab1", [128, NTA, H], dt.bfloat16, kind="Internal")
    out = nc.dram_tensor("out", [128, NTB, H], dt.bfloat16, kind="ExternalOutput")

    with tile.TileContext(nc) as tc:
        with (
            tc.tile_pool(name="const", bufs=1) as cp,
            tc.tile_pool(name="io", bufs=3) as iop,
            tc.tile_pool(name="stg", bufs=2) as stgp,
            tc.tile_pool(name="ps", bufs=2, space="PSUM") as psp,
        ):
            w12_sb = cp.tile([128, KB, H], dt.bfloat16)
            nc.sync.dma_start(w12_sb[:], w12[:])
            vb2_sb = cp.tile([2, H], dt.bfloat16)
            nc.sync.dma_start(vb2_sb[:], vb2[:])
            dis2_sb = cp.tile([128, NTA], dt.float32)
            nc.sync.dma_start(dis2_sb[:], dis2[:])
            disc_sb = cp.tile([128, NTB], dt.float32)
            nc.sync.dma_start(disc_sb[:], disc[:])
            gidx_sb = cp.tile([128, nB // 16], dt.int16)
            nc.sync.dma_start(gidx_sb[:], gidx[:])


            # ---------- stage A: tab1 = dis^2 * (S1 @ t1) ----------
            for p in range(NTA // QL):
                t1_sb = iop.tile([128, QL * B, H], dt.bfloat16, tag="t1")
                nc.sync.dma_start(t1_sb[:], t1[p])
                s1_sb = iop.tile([128, QL * B, 128], dt.float8e4, tag="s1")
                nc.scalar.dma_start(s1_sb[:], s1[p])
                tws = stgp.tile([128, QL, H], dt.bfloat16, tag="tws")
                for h in range(QL):
                    i = p * QL + h
                    psA = psp.tile([128, H], dt.float32, tag="psA")
                    for b in range(B):
                        nc.tensor.matmul(psA[:], s1_sb[:, h * B + b, :],
                                         t1_sb[:, h * B + b, :],
                                         start=(b == 0), stop=(b == B - 1))
                    if i % 2 == 0:
                        nc.vector.tensor_scalar_mul(
                            tws[:, h, :], psA[:], dis2_sb[:, i:i + 1])
                    else:
                        nc.scalar.activation(
                            tws[:, h, :], psA[:], AF.Copy,
                            scale=dis2_sb[:, i:i + 1])
                nc.gpsimd.dma_start(tab1[:, p * QL:(p + 1) * QL, :], tws[:])

            # ------- stages B/C/D fused per owned-tile group ----------
            tab1f = tab1[:].rearrange("p t h -> (p t) h")
            for q in range(NTB // SG):
                s2_sb = iop.tile([128, SG * B, 128], dt.float8e4, tag="s2", bufs=2)
                nc.scalar.dma_start(s2_sb[:], s2[q])
                c12_sb = iop.tile([2, SG * 128], dt.bfloat16, tag="c12", bufs=2)
                nc.scalar.dma_start(c12_sb[:], c12[:, q * SG * 128:(q + 1) * SG * 128])
                g_sb = iop.tile([128, SG * B, H], dt.bfloat16, tag="g", bufs=4)
                for t0 in range(0, SG, 4):
                    tn = min(4, SG - t0)
                    nidx = tn * CAP
                    gb = q * (SG * CAP // 16) + t0 * (CAP // 16)
                    nc.gpsimd.dma_gather(
                        g_sb[:, t0 * B:(t0 + tn) * B, :], tab1f,
                        gidx_sb[:, gb:gb + nidx // 16],
                        nidx, nidx, H, transpose=False)
                ows = stgp.tile([128, SG, H], dt.bfloat16, tag="ows")
                for h in range(SG):
                    j = q * SG + h
                    psC = psp.tile([128, H], dt.float32, tag="psC")
                    for fs in range(KB):
                        for b in range(B):
                            nc.tensor.matmul(
                                psC[:, fs * 128:(fs + 1) * 128],
                                g_sb[:, h * B + b, fs * 128:(fs + 1) * 128],
                                s2_sb[:, h * B + b, :],
                                start=(b == 0), stop=(b == B - 1))
                    zf = iop.tile([128, H], dt.bfloat16, tag="zf")
                    if j % 2 == 0:
                        nc.vector.tensor_copy(zf[:], psC[:])
                    else:
                        nc.scalar.activation(zf[:], psC[:], AF.Copy)
                    psD = psp.tile([128, H], dt.float32, tag="psD")
                    for k in range(KB):
                        nc.tensor.matmul(psD[:], zf[:, k * 128:(k + 1) * 128],
                                         w12_sb[:, k, :],
                                         start=(k == 0), stop=False)
                    nc.tensor.matmul(psD[:], c12_sb[:, h * 128:(h + 1) * 128],
                                     vb2_sb[:], start=False, stop=True)
                    if j % 2 == 0:
                        nc.vector.tensor_scalar_mul(ows[:, h, :], psD[:],
                                                    disc_sb[:, j:j + 1])
                    else:
                        nc.scalar.activation(ows[:, h, :], psD[:], AF.Copy,
                                             scale=disc_sb[:, j:j + 1])
                nc.sync.dma_start(out[:, q * SG:(q + 1) * SG, :], ows[:])
    nc.compile()
    return nc


_CACHE = {}


def kernel(x, edge_index, W1, b1, W2, b2):
    from concourse import bass_utils

    in_maps, dims, perms = _prep(x, edge_index, W1, b1, W2, b2)
    key = tuple(sorted(dims.items()))
    if key not in _CACHE:
        _CACHE[key] = _build(dims)
    nc = _CACHE[key]
    res = bass_utils.run_bass_kernel_spmd(nc, in_maps, core_ids=list(range(M_CORES)))
    NTB = dims["NTB"]
    out = np.empty((N_NODES, H), np.float32)
    for cc in range(M_CORES):
        flat = np.asarray(res.results[cc]["out"]).reshape(128 * NTB, H)
        out[cc * NPC:(cc + 1) * NPC] = flat[perms[cc]].astype(np.float32)
    return out


# revision 12
# speedup vs baseline: 3.9066x; 2.1420x over previous
"""GCN 2-layer message-passing block on 8 Trainium2 NeuronCores — v3.

Same collapsed algebra as v2 (see kernel.py docstring), but the tab1
intermediate is produced directly in stage-B *token order*: each stage-B
token slot (tile, block, partition) owns one tab1 row, so stage B reads
tab1 with plain sequential DMAs (12-15KB descriptors) instead of a
row-granular dma_gather (768B descriptors).  Sources used by k stage-B
tokens are aggregated k times in stage A (~15% extra stage-A tokens).

Program uniformity across the 8 SPMD cores: each core sorts its stage-B
tiles by stage-A workload; the per-column sub-block count profile is the
column-wise max over cores (sorted profiles concentrate, so padding is
small).  Stage-A token streams are loaded in fixed 16-sub-block windows.
"""
import sys

sys.path.insert(0, "/opt/trn_rl_repo")

import numpy as np
import ml_dtypes

BF16 = ml_dtypes.bfloat16
FP8 = ml_dtypes.float8_e4m3

N_NODES = 100000
N_EDGES = 200000
H = 384
KB = H // 128
M_CORES = 8
NPC = N_NODES // M_CORES
B = 2                 # token blocks per dest tile (bin-packed)
CAP = B * 128         # 256 token slots per tile
SG = 10               # stage-B tiles per load/store group
WG = 2 * SG           # stage-A columns per tab1 write group
LWIN = 16             # stage-A sub-blocks per load window


def _ffdpack(sizes):
    """Bin-pack dests into tiles of <=128 dests and <=CAP tokens (largest
    first).  Returns (tile_of, slot_of, tok_off, ntiles)."""
    n = sizes.size
    nb = max(int(np.ceil(sizes.sum() / (CAP - 2))), int(np.ceil(n / 128)))
    while True:
        rem_tok = np.full(nb, CAP, np.int64)
        rem_cnt = np.full(nb, 128, np.int64)
        tile_of = np.full(n, -1, np.int64)
        ok = True
        for s in range(int(sizes.max()), -1, -1):
            items = np.nonzero(sizes == s)[0]
            ii = 0
            while ii < items.size:
                elig = np.nonzero((rem_tok >= s) & (rem_cnt > 0))[0]
                if elig.size == 0:
                    ok = False
                    break
                elig = elig[np.argsort(-rem_tok[elig], kind="stable")]
                take = min(items.size - ii, elig.size)
                sel = elig[:take]
                tile_of[items[ii:ii + take]] = sel
                rem_tok[sel] -= s
                rem_cnt[sel] -= 1
                ii += take
            if not ok:
                break
        if ok:
            break
        nb += 1
    order = np.lexsort((np.arange(n), tile_of))
    slot_of = np.empty(n, np.int64)
    tok_off = np.empty(n, np.int64)
    cnt = np.zeros(nb, np.int64)
    tok = np.zeros(nb, np.int64)
    for i in order:
        t = tile_of[i]
        slot_of[i] = cnt[t]
        tok_off[i] = tok[t]
        cnt[t] += 1
        tok[t] += sizes[i]
    return tile_of, slot_of, tok_off, nb


def _pm_groups(tokens, ntiles, width, dtype, Q):
    """[ntiles*CAP, width] -> group-major [ntiles//Q, 128, Q*B, width]."""
    t = tokens.reshape(ntiles // Q, Q, B, 128, width)
    return np.ascontiguousarray(t.transpose(0, 3, 1, 2, 4).reshape(
        ntiles // Q, 128, Q * B, width)).astype(dtype)


def _core_meta(row, col, cc, indeg_col):
    """Stage-B packing + program ordering for one core."""
    em = (col // NPC) == cc
    er, ec = row[em], col[em]
    dlo = ec - cc * NPC
    szB = np.bincount(dlo, minlength=NPC)
    tileB, slotB_d, tokoffB, ntB = _ffdpack(szB)

    offB = np.zeros(NPC + 1, np.int64)
    np.cumsum(szB, out=offB[1:])
    orderB = np.argsort(dlo, kind="stable")
    rankB = np.arange(dlo.size) - offB[dlo[orderB]]
    dstB = dlo[orderB]
    slotB = tileB[dstB] * CAP + tokoffB[dstB] + rankB  # phys token slot
    tsrc = np.full(ntB * CAP, -1, np.int64)
    tsrc[slotB] = er[orderB]
    tdsl = np.full(ntB * CAP, -1, np.int64)
    tdsl[slotB] = slotB_d[dstB]

    # stage-A workload per physical column (128 token slots)
    wtok = np.where(tsrc >= 0, indeg_col[np.maximum(tsrc, 0)], 0)
    cnt_phys = wtok.reshape(ntB * 2, 128).sum(1)

    # sort tiles by total stage-A workload, heavier block first in a tile
    wt_tile = cnt_phys.reshape(ntB, 2).sum(1)
    tile_order = np.argsort(-wt_tile, kind="stable")  # prog slot m -> phys
    blk_first = np.argmax(cnt_phys.reshape(ntB, 2), axis=1)  # heavier blk
    return dict(ntB=ntB, tileB=tileB, slotB_d=slotB_d, tsrc=tsrc, tdsl=tdsl,
                cnt_phys=cnt_phys, tile_order=tile_order, blk_first=blk_first)


def _prep(x, edge_index, W1, b1, W2, b2):
    row = np.asarray(edge_index[0], dtype=np.int64)
    col = np.asarray(edge_index[1], dtype=np.int64)
    xf = np.asarray(x, dtype=np.float64)

    deg = np.bincount(row, minlength=N_NODES).astype(np.float64)
    dis = deg ** -0.5
    a = np.bincount(col, weights=dis[row], minlength=N_NODES)
    cvec = dis * a
    c2 = dis * np.bincount(col, weights=(dis * cvec)[row], minlength=N_NODES)
    W12 = np.asarray(W2, np.float64) @ np.asarray(W1, np.float64)
    vv = np.asarray(W2, np.float64) @ np.asarray(b1, np.float64)
    xt = (dis[:, None] * xf).astype(BF16)

    # CSR of in-edges keyed by destination (col): stage-A edge lists
    indeg_col = np.bincount(col, minlength=N_NODES)
    order_c = np.argsort(col, kind="stable")
    row_by_col = row[order_c]
    coff = np.zeros(N_NODES + 1, np.int64)
    np.cumsum(indeg_col, out=coff[1:])

    metas = [_core_meta(row, col, cc, indeg_col) for cc in range(M_CORES)]
    NTB = max(m["ntB"] for m in metas)
    NTB += (-NTB) % SG
    NCOL = NTB * 2

    # per-core per-program-column sub-block counts -> global max profile
    def prog_cnt(m):
        cnt = np.zeros(NCOL, np.int64)
        ntB = m["ntB"]
        cp = m["cnt_phys"].reshape(ntB, 2)
        bf = m["blk_first"]
        ordered = np.stack([cp[np.arange(ntB), bf],
                            cp[np.arange(ntB), 1 - bf]], axis=1)
        cnt[:2 * ntB] = ordered[m["tile_order"]].reshape(-1)
        return cnt

    BA = np.maximum(1, -(-np.stack([prog_cnt(m) for m in metas]).max(0) // 128))
    colbase = np.zeros(NCOL + 1, np.int64)
    np.cumsum(BA, out=colbase[1:])
    TOT = int(colbase[-1])
    TOT += (-TOT) % LWIN

    w12 = np.ascontiguousarray(
        W12.T.astype(BF16).reshape(KB, 128, H).transpose(1, 0, 2))
    vrow = vv.astype(BF16).reshape(1, H)
    b2r = np.asarray(b2, dtype=BF16).reshape(1, H)

    in_maps = []
    perms = []
    for cc, m in enumerate(metas):
        ntB = m["ntB"]
        # program tile slot of each phys tile / block order per tile
        m_of_phys = np.empty(ntB, np.int64)
        m_of_phys[m["tile_order"]] = np.arange(ntB)
        bf = m["blk_first"]

        # program token slot for each phys token slot
        phys = np.arange(ntB * CAP)
        ptile = phys // CAP
        pblk = (phys % CAP) // 128
        ppar = phys % 128
        # block position under reorder: 0 if pblk == bf[ptile] else 1
        bpos = (pblk != bf[ptile]).astype(np.int64)
        pcol = m_of_phys[ptile] * 2 + bpos
        # token entries in program order
        tsrc_p = np.full(NCOL * 128, -1, np.int64)
        tdsl_p = np.full(NCOL * 128, -1, np.int64)
        tsrc_p[pcol * 128 + ppar] = m["tsrc"]
        tdsl_p[pcol * 128 + ppar] = m["tdsl"]

        valid = tsrc_p >= 0
        srcs = np.maximum(tsrc_p, 0)
        cnts = np.where(valid, indeg_col[srcs], 0)

        # expand: one entry per stage-A token
        tot_tok = int(cnts.sum())
        ent = np.repeat(np.arange(NCOL * 128), cnts)   # program token entry
        ecol = ent // 128
        epar = ent % 128                                # dest slot in column
        estart = np.repeat(coff[srcs], cnts)
        erank = np.arange(tot_tok) - np.repeat(np.concatenate(
            ([0], np.cumsum(cnts)))[:-1], cnts)
        erows = row_by_col[estart + erank]
        # position within the column's token stream
        ccnt = cnts.reshape(NCOL, 128).sum(1)
        cstart = np.zeros(NCOL + 1, np.int64)
        np.cumsum(ccnt, out=cstart[1:])
        k = np.arange(tot_tok) - np.repeat(cstart[:-1], ccnt)
        sub_abs = colbase[ecol] + k // 128
        part = k % 128

        t1 = np.zeros((128, TOT, H), dtype=BF16)
        t1[part, sub_abs] = xt[erows]
        s1 = np.zeros((128, TOT, 128), dtype=FP8)
        s1[part, sub_abs, epar] = 1.0

        dis2 = np.zeros((128, NCOL), dtype=np.float32)
        dis2[np.arange(NCOL * 128) % 128, np.arange(NCOL * 128) // 128] = \
            np.where(valid, (dis[srcs] ** 2), 0.0).astype(np.float32)

        # stage-B one-hot in program token order
        s2 = np.zeros((NCOL * 128, 128), dtype=np.float32)
        vi = np.nonzero(valid)[0]
        s2[vi, tdsl_p[vi]] = 1.0

        # owned-node indices in program tile numbering
        mt = m_of_phys[m["tileB"]]
        pown = mt * 128 + m["slotB_d"]
        pout = m["slotB_d"] * NTB + mt

        disc = np.zeros((128, NTB), dtype=np.float32)
        dd = dis[cc * NPC:(cc + 1) * NPC]
        disc[m["slotB_d"], mt] = dd.astype(np.float32)
        c12 = np.zeros((2, NTB * 128), dtype=BF16)
        c12[0, pown] = (c2[cc * NPC:(cc + 1) * NPC] / dd).astype(BF16)
        c12[1, pown] = (cvec[cc * NPC:(cc + 1) * NPC] / dd).astype(BF16)

        in_maps.append({
            "t1": t1, "s1": s1, "dis2": dis2,
            "s2": _pm_groups(s2, NTB, 128, FP8, SG),
            "disc": disc, "c12": c12,
            "w12": w12, "vb2": np.concatenate([vrow, b2r], axis=0),
        })
        perms.append(pout)
    return in_maps, dict(NTB=NTB, BA=tuple(int(b) for b in BA), TOT=TOT), perms


def _build(dims):
    import concourse.bass as bass
    import concourse.bacc as bacc
    import concourse.mybir as mybir
    import concourse.tile as tile

    dt = mybir.dt
    AF = mybir.ActivationFunctionType
    NTB, BA, TOT = dims["NTB"], dims["BA"], dims["TOT"]
    NCOL = NTB * 2
    ND = NTB * 128
    colbase = np.zeros(NCOL + 1, np.int64)
    np.cumsum(BA, out=colbase[1:])
    NW = TOT // LWIN

    nc = bacc.Bacc(None, target_bir_lowering=False)
    t1 = nc.dram_tensor("t1", [128, TOT, H], dt.bfloat16, kind="ExternalInput")
    s1 = nc.dram_tensor("s1", [128, TOT, 128], dt.float8e4, kind="ExternalInput")
    dis2 = nc.dram_tensor("dis2", [128, NCOL], dt.float32, kind="ExternalInput")
    s2 = nc.dram_tensor("s2", [NTB // SG, 128, SG * B, 128], dt.float8e4, kind="ExternalInput")
    disc = nc.dram_tensor("disc", [128, NTB], dt.float32, kind="ExternalInput")
    c12 = nc.dram_tensor("c12", [2, ND], dt.bfloat16, kind="ExternalInput")
    w12 = nc.dram_tensor("w12", [128, KB, H], dt.bfloat16, kind="ExternalInput")
    vb2 = nc.dram_tensor("vb2", [2, H], dt.bfloat16, kind="ExternalInput")
    tab1 = nc.dram_tensor("tab1", [128, NCOL, H], dt.bfloat16, kind="Internal")
    out = nc.dram_tensor("out", [128, NTB, H], dt.bfloat16, kind="ExternalOutput")

    with tile.TileContext(nc) as tc:
        with (
            tc.tile_pool(name="const", bufs=1) as cp,
            tc.tile_pool(name="io", bufs=3) as iop,
            tc.tile_pool(name="stg", bufs=2) as stgp,
            tc.tile_pool(name="ps", bufs=2, space="PSUM") as psp,
        ):
            w12_sb = cp.tile([128, KB, H], dt.bfloat16)
            nc.sync.dma_start(w12_sb[:], w12[:])
            vb2_sb = cp.tile([2, H], dt.bfloat16)
            nc.sync.dma_start(vb2_sb[:], vb2[:])
            dis2_sb = cp.tile([128, NCOL], dt.float32)
            nc.sync.dma_start(dis2_sb[:], dis2[:])
            disc_sb = cp.tile([128, NTB], dt.float32)
            nc.sync.dma_start(disc_sb[:], disc[:])

            # ---------- stage A: tab1 rows in stage-B token order ----------
            wins = {}

            def ensure(lw):
                while ensure.hi < min(lw + 1, NW - 1) or ensure.hi < lw:
                    n = ensure.hi + 1
                    tt = iop.tile([128, LWIN, H], dt.bfloat16, tag="t1")
                    nc.sync.dma_start(tt[:], t1[:, n * LWIN:(n + 1) * LWIN, :])
                    ss = iop.tile([128, LWIN, 128], dt.float8e4, tag="s1")
                    nc.scalar.dma_start(ss[:], s1[:, n * LWIN:(n + 1) * LWIN, :])
                    wins[n] = (tt, ss)
                    wins.pop(n - 3, None)
                    ensure.hi = n
            ensure.hi = -1

            for wgi in range(NCOL // WG):
                tws = stgp.tile([128, WG, H], dt.bfloat16, tag="tws")
                for h in range(WG):
                    c = wgi * WG + h
                    nsub = BA[c]
                    base = int(colbase[c])
                    psA = psp.tile([128, H], dt.float32, tag="psA")
                    for s in range(nsub):
                        ab = base + s
                        lw = ab // LWIN
                        ensure(lw)
                        tt, ss = wins[lw]
                        off = ab % LWIN
                        nc.tensor.matmul(psA[:], ss[:, off, :], tt[:, off, :],
                                         start=(s == 0), stop=(s == nsub - 1))
                    if c % 2 == 0:
                        nc.vector.tensor_scalar_mul(
                            tws[:, h, :], psA[:], dis2_sb[:, c:c + 1])
                    else:
                        nc.scalar.activation(
                            tws[:, h, :], psA[:], AF.Copy,
                            scale=dis2_sb[:, c:c + 1])
                nc.gpsimd.dma_start(tab1[:, wgi * WG:(wgi + 1) * WG, :], tws[:])

            # ------- stages B/C/D fused per owned-tile group ----------
            for q in range(NTB // SG):
                s2_sb = iop.tile([128, SG * B, 128], dt.float8e4, tag="s2")
                nc.scalar.dma_start(s2_sb[:], s2[q])
                c12_sb = iop.tile([2, SG * 128], dt.bfloat16, tag="c12")
                nc.scalar.dma_start(c12_sb[:], c12[:, q * SG * 128:(q + 1) * SG * 128])
                g_sb = iop.tile([128, SG * B, H], dt.bfloat16, tag="g")
                nc.gpsimd.dma_start(g_sb[:], tab1[:, q * SG * B:(q + 1) * SG * B, :])
                ows = stgp.tile([128, SG, H], dt.bfloat16, tag="ows")
                for h in range(SG):
                    j = q * SG + h
                    psC = psp.tile([128, H], dt.float32, tag="psC")
                    for fs in range(KB):
                        for b in range(B):
                            nc.tensor.matmul(
                                psC[:, fs * 128:(fs + 1) * 128],
                                g_sb[:, h * B + b, fs * 128:(fs + 1) * 128],
                                s2_sb[:, h * B + b, :],
                                start=(b == 0), stop=(b == B - 1))
                    zf = iop.tile([128, H], dt.bfloat16, tag="zf")
                    if j % 2 == 0:
                        nc.vector.tensor_copy(zf[:], psC[:])
                    else:
                        nc.scalar.activation(zf[:], psC[:], AF.Copy)
                    psD = psp.tile([128, H], dt.float32, tag="psD")
                    for k in range(KB):
                        nc.tensor.matmul(psD[:], zf[:, k * 128:(k + 1) * 128],
                                         w12_sb[:, k, :],
                                         start=(k == 0), stop=False)
                    nc.tensor.matmul(psD[:], c12_sb[:, h * 128:(h + 1) * 128],
                                     vb2_sb[:], start=False, stop=True)
                    if j % 2 == 0:
                        nc.vector.tensor_scalar_mul(ows[:, h, :], psD[:],
                                                    disc_sb[:, j:j + 1])
                    else:
                        nc.scalar.activation(ows[:, h, :], psD[:], AF.Copy,
                                             scale=disc_sb[:, j:j + 1])
                nc.sync.dma_start(out[:, q * SG:(q + 1) * SG, :], ows[:])
    nc.compile()
    return nc


_CACHE = {}


def _cache_key(dims):
    return (dims["NTB"], dims["TOT"], dims["BA"])


def kernel(x, edge_index, W1, b1, W2, b2):
    from concourse import bass_utils

    in_maps, dims, perms = _prep(x, edge_index, W1, b1, W2, b2)
    key = _cache_key(dims)
    if key not in _CACHE:
        _CACHE[key] = _build(dims)
    nc = _CACHE[key]
    res = bass_utils.run_bass_kernel_spmd(nc, in_maps, core_ids=list(range(M_CORES)))
    NTB = dims["NTB"]
    out = np.empty((N_NODES, H), np.float32)
    for cc in range(M_CORES):
        flat = np.asarray(res.results[cc]["out"]).reshape(128 * NTB, H)
        out[cc * NPC:(cc + 1) * NPC] = flat[perms[cc]].astype(np.float32)
    return out
